# revision 1
# baseline (speedup 1.0000x reference)
"""LorentzConv2d Trainium2 kernel (v2: bf16 box matmuls, batched field ops).

Full-input contract: kernel(x=[8,56,56,64], kernels=[64,64]) -> [8,56,56,64].
Data-parallel over batch: one image per NeuronCore (8 cores).

Per-core algorithm (all on a zero-padded 58x58 grid, linearized p = 58*gh+gw):
  u[p,o]   = sum_c x[p,c] * g_c * kernels[o,c]   (PE matmul; g = (+1,-1..-1))
  sx[p]    = sum_{c>=1} x[p,c]                   (extra matmul column)
  D[p,o]   = acosh(max(u, 1+eps))^2 = ln(u + sqrt(u^2-1))^2   (ACT/DVE)
  G[p,d]   = <x[p], x[p+d]>_L  for the 12 positive window offsets d (DVE/GPSIMD)
  Q[l,o]   = -box3x3(D^2)[l] + 2*sum_d boxB(d)( D * shift_d(D) * G_d )[l]
  S1[l,o]  = box3x3(sx * D)[l]
  out_o    = (S1/63) / sqrt(clip(|Q|,eps))  (o>=1);  out_0 = sqrt(1 + sum out_o^2)
All box sums are banded-Toeplitz matmuls on the PE accumulating in PSUM
(bf16 fields/bands, fp32 accumulation; band values 0/±1/2 are bf16-exact).
"""

import os
import numpy as np

import concourse.bass as bass
import concourse.bacc as bacc
import concourse.tile as tile
from concourse import mybir
from concourse.bass_utils import run_bass_kernel_spmd

F32 = mybir.dt.float32
BF16 = mybir.dt.bfloat16
AF = mybir.ActivationFunctionType
OP = mybir.AluOpType

# geometry
H = W = 56
C = 64
O = 64
GH = GW = 58              # padded grid
NG = GH * GW              # 3364
NT = 27                   # pixel tiles of 128
NP = NT * 128             # 3456 compute pixels (grid + tail)
GUARD = 128               # top guard rows in padded DRAM images
NPAD = GUARD + NP + 128   # 3712 rows in xpad/dpad
ACOSH_EPS = 1e-7
EPS = 1e-8

# the 12 positive window-pair offsets (dh, dw), linear = 58*dh+dw
DELTAS = [(0, 1), (0, 2), (1, -2), (1, -1), (1, 0), (1, 1), (1, 2),
          (2, -2), (2, -1), (2, 0), (2, 1), (2, 2)]
ND = len(DELTAS)


def _interval(d):
    return range(max(-1, -1 - d), min(1, 1 - d) + 1)


def _build_passes():
    """Each pass: (name, delta_index_or_None, coeff, box_offsets, target)."""
    box33 = [58 * a + b for a in (-1, 0, 1) for b in (-1, 0, 1)]
    passes = [("diag", None, -1.0, box33, "q")]
    for di, (dh, dw) in enumerate(DELTAS):
        box = [58 * a + b for a in _interval(dh) for b in _interval(dw)]
        passes.append((f"d{di}", di, 2.0, box, "q"))
    passes.append(("s1", None, 1.0, box33, "s"))
    return passes


def _build_bands(passes):
    """Toeplitz band matrices. For pass and side j in {-1,0,1}:
    T[i, m] = coeff if (128*j + i - m) in box else 0.
    Returns (bands [NB,128,128], sides: per-pass list of (j, band_index))."""
    mats = []
    sides = []
    for (_, _, coeff, box, _) in passes:
        bs = set(box)
        plist = []
        for j in (-1, 0, 1):
            T = np.zeros((128, 128), dtype=np.float32)
            for t in bs:
                d = t - 128 * j
                if -127 <= d <= 127:
                    idx = np.arange(max(0, d), 128 + min(0, d))
                    T[idx, idx - d] = coeff
            if np.any(T):
                plist.append((j, len(mats)))
                mats.append(T)
        sides.append(plist)
    return np.stack(mats), sides


PASSES = _build_passes()
BANDS, PASS_SIDES = _build_bands(PASSES)
NB = BANDS.shape[0]


def build_nc():
    nc = bacc.Bacc(None)
    x_in = nc.declare_dram_parameter("x", [H * W, C], F32, isOutput=False)
    gk_in = nc.declare_dram_parameter("gk_ext", [C, O + 1], F32, isOutput=False)
    bands_in = nc.declare_dram_parameter("bands", [NB, 128, 128], BF16,
                                         isOutput=False)
    id_in = nc.declare_dram_parameter("ident", [128, 128], F32, isOutput=False)
    out_ext = nc.declare_dram_parameter("out", [H * W, O], F32, isOutput=True)

    def tiled(dram_ap, row0, ntile=NT):
        """DRAM rows [row0, row0+128*ntile) viewed as [128, ntile, 64]."""
        return dram_ap[row0:row0 + 128 * ntile, :].rearrange(
            "(t p) c -> p t c", p=128)

    with tile.TileContext(nc) as tc:
        with (
            tc.tile_pool(name="dram", bufs=1, space="DRAM") as dpool,
            tc.tile_pool(name="singles", bufs=1) as sg,
            tc.tile_pool(name="pp", bufs=1) as pp,
        ):
            xpad = dpool.tile([NPAD, C], F32)
            xpad16 = dpool.tile([NPAD, C], BF16)
            dpad16 = dpool.tile([NPAD, O], BF16)
            opad = dpool.tile([NP, O], F32)

            # ---- constants into SBUF
            gk_sb = sg.tile([C, O + 1], F32)
            nc.sync.dma_start(out=gk_sb[:], in_=gk_in[:])
            id_sb = sg.tile([128, 128], F32)
            nc.sync.dma_start(out=id_sb[:], in_=id_in[:])
            bands_sb = sg.tile([128, NB, 128], BF16)
            nc.sync.dma_start(out=bands_sb[:],
                              in_=bands_in.rearrange("b p m -> p b m"))

            zsb = sg.tile([128, C], F32)
            nc.vector.memset(zsb[:], 0.0)
            zsb16 = sg.tile([128, C], BF16)
            nc.vector.memset(zsb16[:], 0.0)
            cneg1 = sg.tile([128, 1], F32)
            nc.vector.memset(cneg1[:], -1.0)

            # ---- zero-fill pads (one broadcast DMA each), interior overwrite
            nc.sync.dma_start(
                out=tiled(xpad, 0, NPAD // 128),
                in_=zsb[:].unsqueeze(1).to_broadcast([128, NPAD // 128, C]))
            nc.sync.dma_start(
                out=tiled(dpad16, 0, NPAD // 128),
                in_=zsb16[:].unsqueeze(1).to_broadcast([128, NPAD // 128, O]))
            nc.scalar.dma_start(
                out=tiled(xpad16, 0, NPAD // 128),
                in_=zsb16[:].unsqueeze(1).to_broadcast([128, NPAD // 128, C]))
            g0 = GUARD
            nc.sync.dma_start(
                out=xpad[g0 + GW:g0 + 57 * GW, :].rearrange(
                    "(h w) c -> h w c", w=GW)[:, 1:57, :],
                in_=x_in.rearrange("(h w) c -> h w c", w=W))

            # persistent fields
            x_sb = sg.tile([128, NT, C], F32)
            nc.sync.dma_start(out=x_sb[:], in_=tiled(xpad, GUARD))
            d16 = sg.tile([128, NT, O], BF16)
            x16 = sg.tile([128, NT, C], BF16)
            gx16 = sg.tile([128, NT, C], BF16)
            sx_sb = sg.tile([128, NT], F32)
            sx16 = sg.tile([128, NT], BF16)
            nc.scalar.copy(x16[:], x_sb[:])
            nc.sync.dma_start(out=tiled(xpad16, GUARD), in_=x16[:])
            nc.vector.tensor_copy(gx16[:], x16[:])
            nc.vector.tensor_scalar_mul(gx16[:, :, 0], gx16[:, :, 0], -1.0)
            NBAT = ND // 2
            g16b = [sg.tile([128, 2, NT], BF16, tag=f"g16b{i}", name=f"g16b{i}")
                    for i in range(NBAT)]

            # ================= phase A: u, sx, dists =================
            with (
                tc.tile_pool(name="psA", bufs=1, space="PSUM") as psA,
                tc.tile_pool(name="psT", bufs=3, space="PSUM") as psT,
                tc.tile_pool(name="sbA", bufs=1) as sbA,
            ):
                xT = sbA.tile([64, NT, 128], F32)
                # 4 PSUM pieces of <=7 tiles each so every matmul output stays
                # inside one 2KB PSUM bank (7*65*4B = 1820B)
                ugroups = [(0, 7), (7, 7), (14, 7), (21, 6)]
                psu_g = [psA.tile([128, 7, O + 1], F32, tag=f"psu{i}",
                                  name=f"psu{i}") for i in range(4)]
                for gi, (t0, tn) in enumerate(ugroups):
                    for i in range(tn):
                        tl = t0 + i
                        xt_ps = psT.tile([C, 128], F32)
                        nc.tensor.transpose(xt_ps[:], x_sb[:, tl, :], id_sb[:])
                        nc.scalar.copy(xT[:, tl, :], xt_ps[:])
                        nc.tensor.matmul(psu_g[gi][:, i, :], xT[:, tl, :],
                                         gk_sb[:], start=True, stop=True)
                # batched dists pipeline over all tiles
                um = pp.tile([128, NT, O], F32, tag="big0", name="um")
                for gi, (t0, tn) in enumerate(ugroups):
                    nc.vector.tensor_scalar_max(um[:, t0:t0 + tn, :],
                                                psu_g[gi][:, :tn, 0:O],
                                                1.0 + ACOSH_EPS)
                    nc.scalar.copy(sx_sb[:, t0:t0 + tn], psu_g[gi][:, :tn, O])
                nc.vector.tensor_copy(sx16[:], sx_sb[:])
                sq = pp.tile([128, NT, O], F32, tag="big1", name="sq")
                nc.scalar.activation(sq[:], um[:], AF.Square)
                rt = pp.tile([128, NT, O], F32, tag="big2", name="rt")
                nc.scalar.activation(rt[:], sq[:], AF.Sqrt, bias=cneg1[:])
                vv = pp.tile([128, NT, O], F32, tag="big1", name="vv")
                nc.gpsimd.tensor_add(vv[:], um[:], rt[:])
                lnv = pp.tile([128, NT, O], F32, tag="big0", name="lnv")
                nc.scalar.activation(lnv[:], vv[:], AF.Ln)
                nc.scalar.activation(d16[:], lnv[:], AF.Square)
                nc.sync.dma_start(out=tiled(dpad16, GUARD), in_=d16[:])

            # ===== phases B+C: 2-delta batches; G products feed the banded
            # box matmuls; big fused ops to minimize sync overhead =====
            with (
                tc.tile_pool(name="psQ", bufs=1, space="PSUM") as psQ,
                tc.tile_pool(name="psS", bufs=1, space="PSUM") as psS,
            ):
                ps_q = psQ.tile([128, NT, O], F32)
                ps_s = psS.tile([128, NT, O], F32)

                xs_pp = [pp.tile([128, 2, NT, C], BF16, tag=f"xs{i}",
                                 name=f"xs{i}") for i in range(2)]
                tg_pp = [pp.tile([128, 2, NT, C], BF16, tag=f"tg{i}",
                                 name=f"tg{i}") for i in range(2)]
                ds_pp = [pp.tile([128, 2, NT, O], BF16, tag=f"ds{i}",
                                 name=f"ds{i}") for i in range(2)]
                t2_pp = [pp.tile([128, 2, NT, O], BF16, tag=f"t2{i}",
                                 name=f"t2{i}") for i in range(2)]
                f_pp = [pp.tile([128, 2, NT + 2, O], BF16, tag=f"f{i}",
                                name=f"f{i}") for i in range(3)]
                for f in f_pp:
                    nc.vector.memset(f[:, :, 0, :], 0.0)
                    nc.vector.memset(f[:, :, NT + 1, :], 0.0)

                chunks = [(0, 8), (8, 8), (16, 8), (24, 3)]
                n_writes_q = sum(len(PASS_SIDES[pi]) for pi, p in enumerate(PASSES)
                                 if p[4] == "q")
                n_writes_s = sum(len(PASS_SIDES[pi]) for pi, p in enumerate(PASSES)
                                 if p[4] == "s")
                wq = [0] * len(chunks)
                ws = [0] * len(chunks)

                def box_pass(pi, fsub):
                    tgt_kind = PASSES[pi][4]
                    tgt, wcnt, wtot = ((ps_q, wq, n_writes_q) if tgt_kind == "q"
                                       else (ps_s, ws, n_writes_s))
                    for (j, bi) in PASS_SIDES[pi]:
                        for ci, (c0, cw) in enumerate(chunks):
                            nc.tensor.matmul(
                                tgt[:, c0:c0 + cw, :],
                                bands_sb[:, bi, :],
                                fsub[:, 1 + c0 + j:1 + c0 + j + cw, :],
                                start=(wcnt[ci] == 0),
                                stop=(wcnt[ci] == wtot - 1),
                                skip_group_check=True)
                            wcnt[ci] += 1

                # diag + s1 passes first (only need d16/sx16): PE busy early
                f = f_pp[2]
                nc.scalar.activation(f[:, 0, 1:NT + 1, :], d16[:], AF.Square)
                box_pass(0, f[:, 0, :, :])
                nc.vector.tensor_mul(
                    f[:, 1, 1:NT + 1, :], d16[:],
                    sx16[:].unsqueeze(2).to_broadcast([128, NT, O]))
                box_pass(13, f[:, 1, :, :])

                for bi in range(NBAT):
                    d0 = 2 * bi
                    b = bi % 2
                    xs, tg, dsh, t2, f = (xs_pp[b], tg_pp[b], ds_pp[b],
                                          t2_pp[b], f_pp[b])
                    for k in (0, 1):
                        dh, dw = DELTAS[d0 + k]
                        dlin = 58 * dh + dw
                        eng = nc.sync if k == 0 else nc.scalar
                        eng.dma_start(out=xs[:, k, :, :],
                                      in_=tiled(xpad16, GUARD + dlin))
                        eng.dma_start(out=dsh[:, k, :, :],
                                      in_=tiled(dpad16, GUARD + dlin))
                    # --- G pair: tg = gx (bcast) * xs ; reduce over c ---
                    GSP = 9
                    nc.gpsimd.tensor_mul(
                        tg[:, :, :GSP, :], xs[:, :, :GSP, :],
                        gx16[:, :GSP, :].unsqueeze(1).to_broadcast(
                            [128, 2, GSP, C]))
                    nc.vector.tensor_mul(
                        tg[:, :, GSP:, :], xs[:, :, GSP:, :],
                        gx16[:, GSP:, :].unsqueeze(1).to_broadcast(
                            [128, 2, NT - GSP, C]))
                    gb = pp.tile([128, 2, NT], F32, tag="gb", name="gb")
                    nc.vector.tensor_reduce(gb[:], tg[:],
                                            axis=mybir.AxisListType.X, op=OP.add)
                    nc.vector.tensor_copy(g16b[bi][:], gb[:])
                    # --- F pair = D * shift(D) * G (bf16) ---
                    nc.vector.tensor_mul(
                        t2[:], dsh[:],
                        d16[:].unsqueeze(1).to_broadcast([128, 2, NT, O]))
                    nc.vector.tensor_mul(
                        f[:, :, 1:NT + 1, :], t2[:],
                        g16b[bi][:].unsqueeze(3).to_broadcast([128, 2, NT, O]))
                    box_pass(1 + d0, f[:, 0, :, :])
                    box_pass(2 + d0, f[:, 1, :, :])

                # ================= phase D: normalize & emit =================
                ac = pp.tile([128, NT, O], F32, tag="big0", name="ac")
                nc.scalar.activation(ac[:], ps_q[:], AF.Abs)
                cl = pp.tile([128, NT, O], F32, tag="big1", name="cl")
                nc.gpsimd.tensor_scalar_max(cl[:], ac[:], EPS)
                lnc = pp.tile([128, NT, O], F32, tag="big0", name="lnc")
                nc.scalar.activation(lnc[:], cl[:], AF.Ln)
                rr = pp.tile([128, NT, O], F32, tag="big1", name="rr")
                nc.scalar.activation(rr[:], lnc[:], AF.Exp, scale=-0.5)
                osb = pp.tile([128, NT, O], F32, tag="big2", name="osb")
                nc.vector.scalar_tensor_tensor(
                    out=osb[:], in0=ps_s[:], scalar=1.0 / 63.0, in1=rr[:],
                    op0=OP.mult, op1=OP.mult)
                s2 = pp.tile([128, NT, O - 1], F32, tag="big0", name="s2")
                nc.scalar.activation(s2[:], osb[:, :, 1:O], AF.Square)
                red = pp.tile([128, NT], F32, tag="red", name="red")
                nc.vector.tensor_reduce(red[:], s2[:], axis=mybir.AxisListType.X,
                                        op=OP.add)
                nc.scalar.activation(osb[:, :, 0], red[:], AF.Sqrt, bias=1.0)
                nc.sync.dma_start(out=tiled(opad, 0), in_=osb[:])

            # interior extraction (DRAM -> DRAM)
            nc.sync.dma_start(
                out=out_ext.rearrange("(h w) c -> h w c", w=W),
                in_=opad[GW:57 * GW, :].rearrange(
                    "(h w) c -> h w c", w=GW)[:, 1:57, :])
    nc.finalize()
    return nc


_NC_CACHE = None


def _get_nc():
    global _NC_CACHE
    if _NC_CACHE is None:
        _NC_CACHE = build_nc()
    return _NC_CACHE


def host_consts(kernels):
    # u = -l_inner(x,k) = x0*k0 - sum_{c>=1} x_c*k_c ; col O is sum_{c>=1} x_c
    gk_ext = np.zeros((C, O + 1), dtype=np.float32)
    gk_ext[:, :O] = kernels.astype(np.float32).T
    gk_ext[1:, :O] *= -1.0
    gk_ext[1:, O] = 1.0
    return gk_ext


def kernel(x, kernels):
    import ml_dtypes
    x = np.asarray(x, dtype=np.float32)
    kernels = np.asarray(kernels, dtype=np.float32)
    B = x.shape[0]
    assert x.shape == (B, H, W, C) and B == 8, x.shape
    gk_ext = np.ascontiguousarray(host_consts(kernels))
    ident = np.eye(128, dtype=np.float32)
    bands16 = np.ascontiguousarray(BANDS.astype(ml_dtypes.bfloat16))
    nc = _get_nc()
    in_maps = [{
        "x": np.ascontiguousarray(x[i].reshape(H * W, C)),
        "gk_ext": gk_ext,
        "bands": bands16,
        "ident": ident,
    } for i in range(8)]
    res = run_bass_kernel_spmd(nc, in_maps, core_ids=list(range(8)),
                               trace=bool(int(os.environ.get("KTRACE", "0"))))
    if res.exec_time_ns is not None:
        print(f"HW exec time: {res.exec_time_ns} ns")
    out = np.stack([res.results[i]["out"].reshape(H, W, O) for i in range(8)])
    return out.astype(np.float32)



# revision 10
# speedup vs baseline: 1.1100x; 1.1100x over previous
"""LorentzConv2d Trainium2 kernel (v5: fp8 DMA shifts + DoubleRow box).

Full-input contract: kernel(x=[8,56,56,64], kernels=[64,64]) -> [8,56,56,64].
Data-parallel over batch: one image per NeuronCore (8 cores).

Per-core algorithm on the zero-padded 58x58 grid (pixel p = 58*gh+gw, SBUF
layout [128 part, 27 tiles, 64] with pixel = 128*t + part):
  u[p,o]  = sum_c x[p,c] gu_c k[o,c]  (PE bf16; col 64 gives sx = sum_{c>0} x)
  D[p,o]  = acosh(max(u,1+eps))^2 = ln(u+sqrt(u^2-1))^2    (ACT/DVE chain)
  dpad8   = [D8 | x8] fp8 written once to DRAM; the 12 shifted operand pairs
            (pixel+dlin, dlin = 58*dh+dw in [1,118]) come back as plain
            offset DMA reads -> all products are cheap all-SBUF DVE/Pool ops
  G[p]    = sum_c gx[p,c]/512 * xsh[p,c]                   (DVE mul+reduce)
  F8[p,o] = D*Dsh*G -> fp8                                 (DVE/Pool)
  Q/512   = -box33(D^2/512) + 2 sum_d boxB_d(F8)   fp8 DoubleRow box matmuls
            with HALF-SHIFTED output tiles (out pixel = 128*t+64+i) so one
            256-row band (2 planes = field tiles t,t+1) covers offsets
            in [-60,60] in a single matmul per 8-tile chunk
  S1/8    = box33(D*sx/8)  (same machinery, separate PSUM, phase A)
  out_o   = (S1_psum*8/63) / sqrt(512*max(-Q_psum, 1e-8/512)) (o>0)
  out_0   = sqrt(1 + sum_o out_o^2)
fp8 fields are scaled to stay under the trn fp8e4 max-finite 240; the scales
fold exactly into the final constants. Validated end-to-end rel err ~1.3e-3.
"""

import os
import numpy as np

import bass_rust
import concourse.bass as bass
import concourse.bacc as bacc
import concourse.tile as tile
from concourse import mybir
from concourse.bass_utils import run_bass_kernel_spmd

F32 = mybir.dt.float32
BF16 = mybir.dt.bfloat16
FP8 = mybir.dt.float8e4
AF = mybir.ActivationFunctionType
OP = mybir.AluOpType
DR = mybir.MatmulPerfMode.DoubleRow

# geometry
H = W = 56
C = 64
O = 64
GW = 58
NG = GW * GW               # 3364
NT = 27                    # pixel tiles of 128
NP = NT * 128              # 3456 compute pixels
NFT = NT + 2               # field tiles incl. leading/trailing zero tile
NPAD8 = NP + 128           # dpad8 rows (shift guard)
ACOSH_EPS = 1e-7
EPS_Q = 1e-8 / 512.0
OUT_SCALE = 8.0 / (63.0 * 512.0 ** 0.5)

DELTAS = [(0, 1), (0, 2), (1, -2), (1, -1), (1, 0), (1, 1), (1, 2),
          (2, -2), (2, -1), (2, 0), (2, 1), (2, 2)]
ND = len(DELTAS)
NB = 2 + ND                # diag, s1, delta-box bands
BI_DIAG = 0
BI_S1 = 1
BI_BOX0 = 2

CHUNKS = [(0, 8), (8, 8), (16, 8), (24, 3)]
GROUPS = [(0, 7), (7, 7), (14, 7), (21, 6)]


def _interval(d):
    return range(max(-1, -1 - d), min(1, 1 - d) + 1)


def build_bands():
    """Box band matrices as [NB, 128(p), 2(plane), 128(i)] over a 256-pixel
    window starting one half-tile before the half-shifted out tile:
    T[64+i+s, i] = coeff for s in box."""
    b = np.zeros((NB, 256, 128), np.float32)
    box33 = [GW * a + bb for a in (-1, 0, 1) for bb in (-1, 0, 1)]

    def put_box(bi, offs, coeff):
        for i in range(128):
            for s in offs:
                b[bi, 64 + i + s, i] = coeff

    put_box(BI_DIAG, box33, -1.0)
    put_box(BI_S1, box33, 1.0)
    for di, (dh, dw) in enumerate(DELTAS):
        offs = [GW * a + bb for a in _interval(dh) for bb in _interval(dw)]
        put_box(BI_BOX0 + di, offs, 2.0)
    return np.ascontiguousarray(
        b.reshape(NB, 2, 128, 128).transpose(0, 2, 1, 3))


def dr_rhs(field, t0, tn):
    """Overlapping DoubleRow rhs [128, 2, tn, 64] over field tiles
    [t0, t0+tn]: plane k of out-tile t reads field tile t0+t+k."""
    ap = field[:, t0:t0 + tn + 1, :].unsqueeze(1).to_broadcast(
        [128, 2, tn + 1, 64])[:, :, 0:tn, :]
    ap.ap = bass_rust.VecI64Pair(
        [tuple(ap.ap[0]), (64, 2), (64, tn), (1, 64)])
    return ap


def build_nc():
    nc = bacc.Bacc(None)
    x_in = nc.declare_dram_parameter("x", [H * W, C], F32, isOutput=False)
    gk_in = nc.declare_dram_parameter("gk16", [C, O + 1], BF16, isOutput=False)
    bands_in = nc.declare_dram_parameter("bands", [NB, 128, 2, 128], FP8,
                                         isOutput=False)
    id_in = nc.declare_dram_parameter("ident", [128, 128], BF16, isOutput=False)
    out_ext = nc.declare_dram_parameter("out", [H * W, O], F32, isOutput=True)

    def tiled(dram_ap, ntile=NT):
        return dram_ap[0:128 * ntile, :].rearrange("(t p) c -> p t c", p=128)

    with tile.TileContext(nc) as tc:
        with (
            tc.tile_pool(name="dram", bufs=1, space="DRAM") as dpool,
            tc.tile_pool(name="singles", bufs=1) as sg,
            tc.tile_pool(name="pp", bufs=1) as pp,
            tc.tile_pool(name="shp", bufs=3) as shp,
            tc.tile_pool(name="wk", bufs=2) as wk,
        ):
            xpad = dpool.tile([NP, C], F32)
            dpad8 = dpool.tile([NPAD8, 2 * C], FP8)
            opad = dpool.tile([NP, O], F32)

            # ---- constants into SBUF
            gk_sb = sg.tile([C, O + 1], BF16)
            nc.sync.dma_start(out=gk_sb[:], in_=gk_in[:])
            id_sb = sg.tile([128, 128], BF16)
            nc.sync.dma_start(out=id_sb[:], in_=id_in[:])
            bands_sb = sg.tile([128, NB, 2, 128], FP8)
            nc.sync.dma_start(out=bands_sb[:],
                              in_=bands_in.rearrange("b p two m -> p b two m"))
            zsb = sg.tile([128, 2 * C], F32)
            nc.vector.memset(zsb[:], 0.0)
            z8 = sg.tile([128, 2 * C], FP8)
            nc.vector.memset(z8[:], 0.0)
            cneg1 = sg.tile([128, 1], F32)
            nc.vector.memset(cneg1[:], -1.0)

            # ---- xpad staging (pipelined in 4 row bands) + reload
            # zeros: top [0,59), bottom [3305,3456), inter-row pads
            nc.sync.dma_start(out=xpad[0:59, :], in_=zsb[0:59, 0:C])
            nc.scalar.dma_start(out=xpad[3305:3433, :], in_=zsb[:, 0:C])
            nc.scalar.dma_start(out=xpad[3433:3456, :], in_=zsb[0:23, 0:C])
            nc.scalar.dma_start(
                out=xpad[115:115 + 55 * GW, :].rearrange(
                    "(h r) c -> h r c", r=GW)[:, 0:2, :],
                in_=zsb[0:55, 0:C].unsqueeze(1).to_broadcast([55, 2, C]))
            # interior in 4 grid-row bands; reload tile-groups as they land
            x_sb = sg.tile([128, NT, C], F32)
            hb = [(0, 15), (15, 15), (30, 15), (45, 11)]
            for bi, (h0, hn) in enumerate(hb):
                r0 = 59 + GW * h0
                nc.sync.dma_start(
                    out=xpad[r0:r0 + hn * GW, :].rearrange(
                        "(h r) c -> h r c", r=GW)[:, 0:56, :],
                    in_=x_in[56 * h0:56 * (h0 + hn), :].rearrange(
                        "(h w) c -> h w c", w=W))
            for g, (t0, tn) in enumerate(GROUPS):
                nc.sync.dma_start(
                    out=x_sb[:, t0:t0 + tn, :],
                    in_=xpad[128 * t0:128 * (t0 + tn), :].rearrange(
                        "(t p) c -> p t c", p=128))

            # ---- persistent fields
            x16 = sg.tile([128, NT, C], BF16)
            gx16q = sg.tile([128, NT, C], BF16)   # g*x/512
            dx8 = sg.tile([128, NT, 2 * C], FP8)  # [D8 | x8]
            D16 = sg.tile([128, NT, O], BF16)
            Fd8 = sg.tile([128, NFT, O], FP8)
            Fs8 = sg.tile([128, NFT, O], FP8)
            for f in (Fd8, Fs8):
                nc.vector.memset(f[:, 0, :], 0.0)
                nc.vector.memset(f[:, NFT - 1, :], 0.0)
            sx16 = sg.tile([128, NT], BF16)
            S1_16 = sg.tile([128, NT, O], BF16)
            xT16 = sg.tile([64, NT, 128], BF16)

            nc.scalar.copy(x16[:], x_sb[:])
            nc.scalar.activation(gx16q[:, :, 1:C], x16[:, :, 1:C], AF.Copy,
                                 scale=1.0 / 512.0)
            nc.scalar.activation(gx16q[:, :, 0], x16[:, :, 0], AF.Copy,
                                 scale=-1.0 / 512.0)
            nc.vector.tensor_copy(dx8[:, :, C:2 * C], x16[:])

            # ================= phase A: u-matmuls + acosh chain ============
            with (
                tc.tile_pool(name="psT", bufs=3, space="PSUM") as psT,
                tc.tile_pool(name="psU", bufs=1, space="PSUM") as psU,
            ):
                psu_g = [psU.tile([128, 7, O + 1], F32, tag=f"psu{g}",
                                  name=f"psu{g}") for g in range(4)]
                for g, (t0, tn) in enumerate(GROUPS):
                    for i in range(tn):
                        tl = t0 + i
                        tp = psT.tile([64, 128], BF16)
                        nc.tensor.transpose(tp[:], x16[:, tl, :], id_sb[:])
                        nc.vector.tensor_copy(xT16[:, tl, :], tp[:])
                        nc.tensor.matmul(psu_g[g][:, i, :], xT16[:, tl, :],
                                         gk_sb[:], start=True, stop=True)
                um = pp.tile([128, NT, O], F32, tag="big0", name="um")
                sq = pp.tile([128, NT, O], F32, tag="big1", name="sq")
                rt = pp.tile([128, NT, O], F32, tag="big2", name="rt")
                vv = pp.tile([128, NT, O], F32, tag="big3", name="vv")
                lnv = pp.tile([128, NT, O], F32, tag="big4", name="lnv")

                def for_groups(fn):
                    for g, (t0, tn) in enumerate(GROUPS):
                        fn(g, slice(t0, t0 + tn), slice(1 + t0, 1 + t0 + tn),
                           tn)

                for_groups(lambda g, sl, fl, tn: nc.vector.tensor_scalar_max(
                    um[:, sl, :], psu_g[g][:, :tn, 0:O], 1.0 + ACOSH_EPS))
                for_groups(lambda g, sl, fl, tn: nc.scalar.copy(
                    sx16[:, sl], psu_g[g][:, :tn, O]))
                for_groups(lambda g, sl, fl, tn: nc.scalar.activation(
                    sq[:, sl, :], um[:, sl, :], AF.Square))
                for_groups(lambda g, sl, fl, tn: nc.scalar.activation(
                    rt[:, sl, :], sq[:, sl, :], AF.Sqrt, bias=cneg1[:]))
                for_groups(lambda g, sl, fl, tn: nc.gpsimd.tensor_add(
                    vv[:, sl, :], um[:, sl, :], rt[:, sl, :]))
                for_groups(lambda g, sl, fl, tn: nc.scalar.activation(
                    lnv[:, sl, :], vv[:, sl, :], AF.Ln))
                for_groups(lambda g, sl, fl, tn: nc.vector.tensor_mul(
                    D16[:, sl, :], lnv[:, sl, :], lnv[:, sl, :]))
                for_groups(lambda g, sl, fl, tn: nc.vector.tensor_copy(
                    dx8[:, sl, 0:C], D16[:, sl, :]))
                for_groups(lambda g, sl, fl, tn: nc.vector.scalar_tensor_tensor(
                    out=Fd8[:, fl, :], in0=D16[:, sl, :], scalar=1.0 / 512.0,
                    in1=D16[:, sl, :], op0=OP.mult, op1=OP.mult))
                for_groups(lambda g, sl, fl, tn: nc.vector.scalar_tensor_tensor(
                    out=Fs8[:, fl, :], in0=D16[:, sl, :], scalar=0.125,
                    in1=sx16[:, sl].unsqueeze(2).to_broadcast([128, tn, O]),
                    op0=OP.mult, op1=OP.mult))
                # dpad8 staging writes (per group) + guard zeros
                for g, (t0, tn) in enumerate(GROUPS):
                    nc.scalar.dma_start(
                        out=dpad8[128 * t0:128 * (t0 + tn), :].rearrange(
                            "(t p) c -> p t c", p=128),
                        in_=dx8[:, t0:t0 + tn, :])
                nc.scalar.dma_start(out=dpad8[NP:NPAD8, :], in_=z8[:])

            # shifted operand loads (fp8, fat-ish rows), spread across queues
            dxsh = []
            for di, (dh, dw) in enumerate(DELTAS):
                dlin = GW * dh + dw
                t = shp.tile([128, NT, 2 * C], FP8, tag=f"dxsh{di % 3}",
                             name=f"dxsh{di}")
                eng = (nc.sync, nc.scalar)[di % 2]
                eng.dma_start(out=t[:], in_=dpad8[dlin:dlin + NP, :].rearrange(
                    "(t p) c -> p t c", p=128))
                dxsh.append(t)

            # ================= phase A2: S1 box =================
            with tc.tile_pool(name="psS", bufs=1, space="PSUM") as psS:
                ps_s = psS.tile([128, NT, O], F32)
                for (c0, cw) in CHUNKS:
                    nc.tensor.matmul(ps_s[:, c0:c0 + cw, :],
                                     bands_sb[:, BI_S1, :, :],
                                     dr_rhs(Fs8, c0, cw),
                                     start=True, stop=True, perf_mode=DR,
                                     skip_group_check=True)
                    nc.scalar.copy(S1_16[:, c0:c0 + cw, :],
                                   ps_s[:, c0:c0 + cw, :])

            # ================= phase C: deltas =================
            with (
                tc.tile_pool(name="psQ", bufs=1, space="PSUM") as psQp,
                tc.tile_pool(name="f8p", bufs=3) as f8p,
            ):
                ps_q = psQp.tile([128, NT, O], F32)
                wq = [0] * len(CHUNKS)
                NWQ = 1 + ND

                def box_pass(bi, field):
                    for ci, (c0, cw) in enumerate(CHUNKS):
                        nc.tensor.matmul(ps_q[:, c0:c0 + cw, :],
                                         bands_sb[:, bi, :, :],
                                         dr_rhs(field, c0, cw),
                                         start=(wq[ci] == 0),
                                         stop=(wq[ci] == NWQ - 1),
                                         perf_mode=DR, skip_group_check=True)
                        wq[ci] += 1

                box_pass(BI_DIAG, Fd8)

                f8_bufs = []
                for i in range(3):
                    f = f8p.tile([128, NFT, O], FP8, tag=f"f8_{i}",
                                 name=f"f8_{i}")
                    nc.vector.memset(f[:, 0, :], 0.0)
                    nc.vector.memset(f[:, NFT - 1, :], 0.0)
                    f8_bufs.append(f)

                with nc.allow_low_precision(reason="G in bf16 is plenty"):
                    prev = None
                    for d in range(ND):
                        F8f = f8_bufs[d % 3]
                        sh = dxsh[d]
                        gxs = wk.tile([128, NT, C], BF16, tag="gxs",
                                      name=f"gxs{d}")
                        nc.vector.tensor_mul(gxs[:], gx16q[:],
                                             sh[:, :, C:2 * C])
                        G16 = wk.tile([128, NT], BF16, tag="G16",
                                      name=f"G16{d}")
                        nc.vector.tensor_reduce(G16[:], gxs[:],
                                                axis=mybir.AxisListType.X,
                                                op=OP.add)
                        t2 = wk.tile([128, NT, O], BF16, tag="t2",
                                     name=f"t2{d}")
                        nc.gpsimd.tensor_mul(t2[:], D16[:], sh[:, :, 0:C])
                        feng = nc.vector if d % 2 == 0 else nc.gpsimd
                        feng.tensor_mul(
                            F8f[:, 1:NT + 1, :], t2[:],
                            G16[:].unsqueeze(2).to_broadcast([128, NT, O]))
                        if prev is not None:
                            box_pass(BI_BOX0 + d - 1, prev)
                        prev = F8f
                    box_pass(BI_BOX0 + ND - 1, prev)

                # ================= phase D: normalize & emit ===============
                osb = pp.tile([128, NT, O], F32, tag="big0", name="osb")
                negq = pp.tile([128, NT, O], F32, tag="big1", name="nq")
                lncl = pp.tile([128, NT, O], F32, tag="big2", name="lncl")
                rr = pp.tile([128, NT, O], F32, tag="big3", name="rr")
                s2 = pp.tile([128, NT, O - 1], F32, tag="big4", name="s2")
                red = pp.tile([128, NT], F32, tag="red", name="red")
                DCH = ((0, 14), (14, 13))
                for (c0, cw) in DCH:
                    sl = slice(c0, c0 + cw)
                    nc.vector.tensor_scalar(negq[:, sl, :], ps_q[:, sl, :],
                                            -1.0, EPS_Q, op0=OP.mult,
                                            op1=OP.max)
                for (c0, cw) in DCH:
                    sl = slice(c0, c0 + cw)
                    nc.scalar.activation(lncl[:, sl, :], negq[:, sl, :], AF.Ln)
                for (c0, cw) in DCH:
                    sl = slice(c0, c0 + cw)
                    nc.scalar.activation(rr[:, sl, :], lncl[:, sl, :],
                                         AF.Exp, scale=-0.5)
                    nc.vector.scalar_tensor_tensor(
                        out=osb[:, sl, :], in0=S1_16[:, sl, :],
                        scalar=OUT_SCALE, in1=rr[:, sl, :],
                        op0=OP.mult, op1=OP.mult)
                    nc.vector.tensor_mul(s2[:, sl, :], osb[:, sl, 1:O],
                                         osb[:, sl, 1:O])
                    nc.vector.tensor_reduce(red[:, sl], s2[:, sl, :],
                                            axis=mybir.AxisListType.X,
                                            op=OP.add)
                for (c0, cw) in DCH:
                    sl = slice(c0, c0 + cw)
                    nc.scalar.activation(osb[:, sl, 0], red[:, sl], AF.Sqrt,
                                         bias=1.0)
                    nc.sync.dma_start(out=tiled(opad)[:, sl, :],
                                      in_=osb[:, sl, :])

            # interior extraction: out pixel q lives at opad row q + 64
            nc.sync.dma_start(
                out=out_ext.rearrange("(h w) c -> h w c", w=W),
                in_=opad[123:123 + 56 * GW, :].rearrange(
                    "(h r) c -> h r c", r=GW)[:, 0:56, :])
    nc.finalize()
    return nc


_NC_CACHE = None


def _get_nc():
    global _NC_CACHE
    if _NC_CACHE is None:
        _NC_CACHE = build_nc()
    return _NC_CACHE


def host_consts(kernels):
    import ml_dtypes
    gk = np.zeros((C, O + 1), np.float32)
    gk[:, :O] = kernels.astype(np.float32).T
    gk[1:, :O] *= -1.0
    gk[1:, O] = 1.0
    gk16 = np.ascontiguousarray(gk.astype(ml_dtypes.bfloat16))
    bands8 = np.ascontiguousarray(build_bands().astype(ml_dtypes.float8_e4m3fn))
    ident16 = np.ascontiguousarray(np.eye(128).astype(ml_dtypes.bfloat16))
    return gk16, bands8, ident16


def kernel(x, kernels):
    x = np.asarray(x, dtype=np.float32)
    kernels = np.asarray(kernels, dtype=np.float32)
    B = x.shape[0]
    assert x.shape == (B, H, W, C) and B == 8, x.shape
    gk16, bands8, ident16 = host_consts(kernels)
    nc = _get_nc()
    in_maps = [{
        "x": np.ascontiguousarray(x[i].reshape(H * W, C)),
        "gk16": gk16,
        "bands": bands8,
        "ident": ident16,
    } for i in range(8)]
    res = run_bass_kernel_spmd(nc, in_maps, core_ids=list(range(8)),
                               trace=bool(int(os.environ.get("KTRACE", "0"))))
    if res.exec_time_ns is not None:
        print(f"HW exec time: {res.exec_time_ns} ns")
    out = np.stack([res.results[i]["out"].reshape(H, W, O) for i in range(8)])
    return out.astype(np.float32)
